# revision 7
# baseline (speedup 1.0000x reference)
"""Trainium2 Bass kernel for nn_BivariateNormalAttention (fp8 DoubleRow).

Self-contained: takes FULL inputs (B=16), shards batch across 8 NeuronCores
(2 images/core), runs a Bass/Tile kernel per core, gathers [16,8,56,56].

conv1 (512->256) runs in fp8e4m3 with perf_mode=DoubleRow (2x128-ch
contraction per pass, 0.5 cyc/row); conv2 (256->256) the same but with
hi+lo split weights (residual x16) to keep its quantization error out of
the pooled features. Convs 3-5 / fc / attention head run in fp32 exactly
as the f32r baseline. Weights are prescaled by 64 on the host to dodge
fp8 subnormals; the activation stage rescales. conv1 output stays
SBUF-resident in fp8 (no DRAM roundtrip). 8 PSUM banks accumulate 8
strips (4 for split) per LDWEIGHTS so weight loads amortize.
"""
import sys
import numpy as np

for _p in ("/opt/trn_rl_repo", "/root/.axon_site/_ro/trn_rl_repo"):
    if _p not in sys.path:
        sys.path.append(_p)

import concourse.bacc as bacc
import concourse.mybir as mybir
import concourse.tile as tile


F32 = mybir.dt.float32
FP8 = mybir.dt.float8e4
DR = mybir.MatmulPerfMode.DoubleRow

B, C, H, W = 16, 512, 112, 112
OUT, GMM = 8, 4
NCORE = 8
IMG = B // NCORE
HP, WP = H + 2, W + 2            # 114
FLAT = HP * WP                   # 12996
FLATP = 13008                    # FLAT padded to %16
RS = 2                           # strip rows (DR moving limit: 2*228 <= 512)
NSTRIP = H // RS                 # 56
NFREE = RS * WP                  # 228
H2 = W2 = H // 2
SIG2 = float(H) / 2.0
LOGR = float(np.log(3.0))
WS = 64.0                        # weight prescale
RSPLIT = 16.0                    # residual prescale for split weights


def build_nc(r_loop=None, split1=False, split2=False, phases="abc"):
    nc = bacc.Bacc("TRN2", target_bir_lowering=False, debug=False)

    n1 = 2 if split1 else 1
    n2 = 2 if split2 else 1
    x = nc.dram_tensor("x", [IMG, 4, 128, FLAT], FP8, kind="ExternalInput")
    w1t = nc.dram_tensor("w1t", [128, n1, 9, 4, 256], FP8, kind="ExternalInput")
    w2t = nc.dram_tensor("w2t", [128, n2, 9, 2, 256], FP8, kind="ExternalInput")
    w3t = nc.dram_tensor("w3t", [128, 9, 2, 128], F32, kind="ExternalInput")
    w4t = nc.dram_tensor("w4t", [128, 9, 128], F32, kind="ExternalInput")
    w5t = nc.dram_tensor("w5t", [128, 9, 64], F32, kind="ExternalInput")
    wfct = nc.dram_tensor("wfct", [64, 9, 128], F32, kind="ExternalInput")
    b1d = nc.dram_tensor("b1d", [128, 2], F32, kind="ExternalInput")
    b2d = nc.dram_tensor("b2d", [128, 2], F32, kind="ExternalInput")
    b3d = nc.dram_tensor("b3d", [128, 1], F32, kind="ExternalInput")
    b4d = nc.dram_tensor("b4d", [128, 1], F32, kind="ExternalInput")
    b5d = nc.dram_tensor("b5d", [64, 1], F32, kind="ExternalInput")
    selpd = nc.dram_tensor("selpd", [128, 128], F32, kind="ExternalInput")
    selgd = nc.dram_tensor("selgd", [32, 8], F32, kind="ExternalInput")
    negiod = nc.dram_tensor("negiod", [32, 56], F32, kind="ExternalInput")
    cstd = nc.dram_tensor("cstd", [32, 1], F32, kind="ExternalInput")

    out = nc.dram_tensor("out", [IMG, OUT, H2, W2], F32, kind="ExternalOutput")

    GA = 8 // n1                  # strips per weight-load group, conv1
    GB = 8 // n2                  # strips per group, conv2
    GLA = (2 * GA + 2) * WP + 2   # group x-span (elements)
    GLA16 = (GLA + 15) // 16 * 16
    inv_ws = 1.0 / WS
    inv_rs = 1.0 / RSPLIT

    with tile.TileContext(nc) as tc:
        def emit_body():
            with (
                tc.tile_pool(name="persist", bufs=1) as pp,
            ):
                b1 = pp.tile([128, 2], F32)
                b2 = pp.tile([128, 2], F32)
                nc.gpsimd.dma_start(b1[:], b1d[:])
                nc.gpsimd.dma_start(b2[:], b2d[:])
                pacc = [[pp.tile([128, 49], F32, name=f"pacc{i}_{c}", tag=f"pacc{i}_{c}")
                         for c in range(2)] for i in range(IMG)]
                # SBUF-resident conv1 output, fp8, per image
                c1t = [pp.tile([128, 2, FLATP], FP8, name=f"c1_{i}", tag=f"c1_{i}")
                       for i in range(IMG)]
                # zero borders + flat tail once (gpsimd, overlaps compute)
                for i in range(IMG):
                    for co in range(2):
                        v = c1t[i][:, co, 0:FLAT].rearrange(
                            "p (r c) -> p r c", c=WP)
                        nc.gpsimd.memset(v[:, 0:1, :], 0.0)
                        nc.gpsimd.memset(v[:, HP - 1:HP, :], 0.0)
                        nc.gpsimd.memset(v[:, :, 0:1], 0.0)
                        nc.gpsimd.memset(v[:, :, WP - 1:WP], 0.0)
                        nc.gpsimd.memset(c1t[i][:, co, FLAT:FLATP], 0.0)

                # ---------------- phase A: conv1 ----------------
                with (
                    tc.tile_pool(name="w1p", bufs=1) as w1p,
                    tc.tile_pool(name="xa", bufs=3) as xa,
                    tc.tile_pool(name="psa", bufs=1, space="PSUM") as psa,
                ):
                    w1 = w1p.tile([128, n1, 9, 4, 256], FP8)
                    nc.gpsimd.dma_start(w1[:], w1t[:])
                    for img in (range(IMG) if "a" in phases else []):
                        xflat = x[img]
                        for sg in range(NSTRIP // GA):
                            base = RS * GA * sg * WP
                            nload = min(GLA, FLAT - base)
                            xt = xa.tile([128, 4, GLA16], FP8, tag="xg")
                            if nload < GLA16:
                                nc.vector.memset(xt[:, :, nload:GLA16], 0.0)
                            for ci in range(4):
                                nc.gpsimd.dma_start(
                                    xt[:, ci, :nload],
                                    xflat[ci, :, base:base + nload])
                            for co in range(2):
                                pss = [psa.tile([128, NFREE], F32,
                                                name=f"psA{q}", tag=f"psA{q}")
                                       for q in range(8)]
                                k = 0
                                for p in range(2):
                                    for t in range(9):
                                        toff = (t // 3) * WP + t % 3
                                        for sp in range(n1):
                                            lh = w1[:, sp, t, 2 * p:2 * p + 2,
                                                    co * 128:(co + 1) * 128]
                                            nc.tensor.ldweights(lh, perf_mode=DR)
                                            for g in range(GA):
                                                mi = nc.tensor.matmul(
                                                    pss[sp * GA + g][:], lh,
                                                    xt[:, 2 * p:2 * p + 2,
                                                       RS * g * WP + toff:
                                                       RS * g * WP + toff + NFREE],
                                                    start=(k == 0),
                                                    stop=(k == 17),
                                                    perf_mode=DR)
                                                mi.ldweights = False
                                        k += 1
                                for g in range(GA):
                                    if split1:
                                        ea = xa.tile([128, NFREE], F32,
                                                     tag="ea")
                                        nc.vector.tensor_copy(
                                            ea[:], pss[GA + g][:])
                                        nc.vector.scalar_tensor_tensor(
                                            pss[g][:], ea[:], inv_rs,
                                            pss[g][:], mybir.AluOpType.mult,
                                            mybir.AluOpType.add)
                                    s_abs = GA * sg + g
                                    vdst = c1t[img][:, co, 0:FLAT].rearrange(
                                        "p (r c) -> p r c", c=WP)
                                    nc.scalar.activation(
                                        vdst[:, 1 + RS * s_abs:1 + RS * (s_abs + 1),
                                             1:113],
                                        pss[g][:].rearrange(
                                            "p (a b) -> p a b", b=WP)[:, :, 0:112],
                                        mybir.ActivationFunctionType.Relu,
                                        bias=b1[:, co:co + 1], scale=inv_ws)

                # ---------------- phase B: conv2 + avgpool16 ----------------
                with (
                    tc.tile_pool(name="w2p", bufs=1) as w2p,
                    tc.tile_pool(name="ob", bufs=4) as ob,
                    tc.tile_pool(name="psb", bufs=1, space="PSUM") as psb,
                ):
                    w2 = w2p.tile([128, n2, 9, 2, 256], FP8)
                    nc.gpsimd.dma_start(w2[:], w2t[:])
                    for img in (range(IMG) if "b" in phases else []):
                        for c in range(2):
                            nc.vector.memset(pacc[img][c][:], 0.0)
                        for sg in range(NSTRIP // GB):
                            for co in range(2):
                                pss = [psb.tile([128, NFREE], F32,
                                                name=f"psB{q}", tag=f"psB{q}")
                                       for q in range(8)]
                                for t in range(9):
                                    toff = (t // 3) * WP + t % 3
                                    for sp in range(n2):
                                        lh = w2[:, sp, t, 0:2,
                                                co * 128:(co + 1) * 128]
                                        nc.tensor.ldweights(lh, perf_mode=DR)
                                        for g in range(GB):
                                            s_abs = GB * sg + g
                                            off = RS * s_abs * WP + toff
                                            mi = nc.tensor.matmul(
                                                pss[sp * GB + g][:], lh,
                                                c1t[img][:, 0:2,
                                                         off:off + NFREE],
                                                start=(t == 0), stop=(t == 8),
                                                perf_mode=DR)
                                            mi.ldweights = False
                                for g in range(GB):
                                    if split2:
                                        el = ob.tile([128, NFREE], F32,
                                                     tag="el")
                                        nc.vector.tensor_copy(
                                            el[:], pss[GB + g][:])
                                        nc.vector.scalar_tensor_tensor(
                                            pss[g][:], el[:], inv_rs,
                                            pss[g][:], mybir.AluOpType.mult,
                                            mybir.AluOpType.add)
                                    s_abs = GB * sg + g
                                    et = ob.tile([128, RS, WP], F32, tag="et")
                                    nc.scalar.activation(
                                        et[:],
                                        pss[g][:].rearrange(
                                            "p (a b) -> p a b", b=WP),
                                        mybir.ActivationFunctionType.Relu,
                                        bias=b2[:, co:co + 1], scale=inv_ws)
                                    rs_ = ob.tile([128, 7], F32, tag="rs")
                                    nc.vector.reduce_sum(
                                        rs_[:],
                                        et[:, :, 0:112].rearrange(
                                            "p r (g c) -> p g r c", c=16),
                                        axis=mybir.AxisListType.XY)
                                    blk = RS * s_abs // 16
                                    nc.vector.tensor_add(
                                        pacc[img][co][:, blk * 7:(blk + 1) * 7],
                                        pacc[img][co][:, blk * 7:(blk + 1) * 7],
                                        rs_[:])

                # ---------------- phase C: head (same as baseline) ----------
                with (
                    tc.tile_pool(name="wc", bufs=1) as wc,
                    tc.tile_pool(name="hc", bufs=1) as hc,
                    tc.tile_pool(name="att", bufs=1) as attp,
                    tc.tile_pool(name="psc", bufs=1, space="PSUM") as psc,
                ):
                    w3 = wc.tile([128, 9, 2, 128], F32)
                    w4 = wc.tile([128, 9, 128], F32)
                    w5 = wc.tile([128, 9, 64], F32)
                    wfc = wc.tile([64, 9, 128], F32)
                    b3 = wc.tile([128, 1], F32)
                    b4 = wc.tile([128, 1], F32)
                    b5 = wc.tile([64, 1], F32)
                    selp = wc.tile([128, 128], F32)
                    selg = wc.tile([32, 8], F32)
                    negio = wc.tile([32, 56], F32)
                    cst = wc.tile([32, 1], F32)
                    for tdst, tsrc in ((w3, w3t), (w4, w4t), (w5, w5t),
                                       (wfc, wfct), (b3, b3d), (b4, b4d),
                                       (b5, b5d), (selp, selpd), (selg, selgd),
                                       (negio, negiod), (cst, cstd)):
                        nc.gpsimd.dma_start(tdst[:], tsrc[:])

                    for img in (range(IMG) if "c" in phases else []):
                        p3in = []
                        for ci in range(2):
                            pi = hc.tile([128, 83], F32, tag=f"p3in{ci}")
                            nc.vector.memset(pi[:], 0.0)
                            nc.vector.tensor_copy(
                                pi[:, 10:73].rearrange(
                                    "p (a b) -> p a b", b=9)[:, :, 0:7],
                                pacc[img][ci][:].rearrange(
                                    "p (a b) -> p a b", b=7))
                            p3in.append(pi)
                        ps3 = psc.tile([128, 63], F32, tag="ps3")
                        k = 0
                        for ci in range(2):
                            for t in range(9):
                                nc.tensor.matmul(
                                    ps3[:], w3[:, t, ci, :],
                                    p3in[ci][:, (t // 3) * 9 + t % 3:
                                             (t // 3) * 9 + t % 3 + 63],
                                    start=(k == 0), stop=(k == 17))
                                k += 1
                        p4in = hc.tile([128, 83], F32, tag="p4in")
                        nc.vector.memset(p4in[:], 0.0)
                        nc.scalar.activation(
                            p4in[:, 10:73].rearrange(
                                "p (a b) -> p a b", b=9)[:, :, 0:7],
                            ps3[:].rearrange("p (a b) -> p a b", b=9)[:, :, 0:7],
                            mybir.ActivationFunctionType.Relu, bias=b3[:, 0:1])
                        ps4 = psc.tile([128, 63], F32, tag="ps4")
                        for t in range(9):
                            nc.tensor.matmul(
                                ps4[:], w4[:, t, :],
                                p4in[:, (t // 3) * 9 + t % 3:
                                     (t // 3) * 9 + t % 3 + 63],
                                start=(t == 0), stop=(t == 8))
                        c4t = hc.tile([128, 49], F32, tag="c4t")
                        nc.scalar.activation(
                            c4t[:].rearrange("p (a b) -> p a b", b=7),
                            ps4[:].rearrange("p (a b) -> p a b", b=9)[:, :, 0:7],
                            mybir.ActivationFunctionType.Relu, bias=b4[:, 0:1])
                        c4v = c4t[:].rearrange("p (y x) -> p y x", x=7)
                        a1 = hc.tile([128, 7, 3], F32, tag="a1")
                        nc.vector.tensor_add(a1[:], c4v[:, :, 0:5:2],
                                             c4v[:, :, 1:6:2])
                        nc.vector.tensor_add(a1[:], a1[:], c4v[:, :, 2:7:2])
                        a2 = hc.tile([128, 9], F32, tag="a2")
                        a2v = a2[:].rearrange("p (i j) -> p i j", j=3)
                        nc.vector.tensor_add(a2v, a1[:, 0:5:2, :],
                                             a1[:, 1:6:2, :])
                        nc.vector.tensor_add(a2v, a2v, a1[:, 2:7:2, :])
                        p5in = hc.tile([128, 27], F32, tag="p5in")
                        nc.vector.memset(p5in[:], 0.0)
                        nc.vector.tensor_copy(
                            p5in[:, 6:21].rearrange(
                                "p (a b) -> p a b", b=5)[:, :, 0:3],
                            a2[:].rearrange("p (a b) -> p a b", b=3))
                        ps5 = psc.tile([64, 15], F32, tag="ps5")
                        for t in range(9):
                            nc.tensor.matmul(
                                ps5[:], w5[:, t, :],
                                p5in[:, (t // 3) * 5 + t % 3:
                                     (t // 3) * 5 + t % 3 + 15],
                                start=(t == 0), stop=(t == 8))
                        h5 = hc.tile([64, 9], F32, tag="h5")
                        nc.scalar.activation(
                            h5[:].rearrange("p (a b) -> p a b", b=3),
                            ps5[:].rearrange("p (a b) -> p a b", b=5)[:, :, 0:3],
                            mybir.ActivationFunctionType.Relu, bias=b5[:, 0:1])
                        psf = psc.tile([128, 1], F32, tag="psf")
                        for t in range(9):
                            nc.tensor.matmul(psf[:], wfc[:, t, :], h5[:, t:t + 1],
                                             start=(t == 0), stop=(t == 8))
                        sig = hc.tile([128, 1], F32, tag="sig")
                        nc.scalar.activation(sig[:], psf[:],
                                             mybir.ActivationFunctionType.Sigmoid)
                        psl = psc.tile([128, 1], F32, tag="psl")
                        nc.tensor.matmul(psl[:], selp[:], sig[:],
                                         start=True, stop=True)
                        mx = hc.tile([32, 1], F32, tag="mx")
                        my = hc.tile([32, 1], F32, tag="my")
                        nc.vector.tensor_copy(mx[:], psl[0:32])
                        nc.vector.tensor_copy(my[:], psl[32:64])
                        r32 = hc.tile([32, 1], F32, tag="r32")
                        nc.scalar.activation(r32[:], psl[64:96],
                                             mybir.ActivationFunctionType.Exp,
                                             bias=cst[:, 0:1])
                        rho = hc.tile([32, 1], F32, tag="rho")
                        nc.vector.tensor_scalar(rho[:], psl[96:128], -0.8, None,
                                                mybir.AluOpType.add)
                        rr = hc.tile([32, 1], F32, tag="rr")
                        nc.vector.tensor_mul(rr[:], rho[:], rho[:])
                        om = hc.tile([32, 1], F32, tag="om")
                        nc.vector.tensor_scalar(om[:], rr[:], -1.0, 1.0,
                                                mybir.AluOpType.mult,
                                                mybir.AluOpType.add)
                        iom = hc.tile([32, 1], F32, tag="iom")
                        nc.vector.reciprocal(iom[:], om[:])
                        den = hc.tile([32, 1], F32, tag="den")
                        nc.vector.tensor_scalar(den[:], iom[:],
                                                -0.5 / (SIG2 * SIG2), None,
                                                mybir.AluOpType.mult)
                        ai = hc.tile([32, 1], F32, tag="ai")
                        nc.vector.tensor_mul(ai[:], den[:], r32[:])
                        ir = hc.tile([32, 1], F32, tag="ir")
                        nc.vector.reciprocal(ir[:], r32[:])
                        bj = hc.tile([32, 1], F32, tag="bj")
                        nc.vector.tensor_mul(bj[:], den[:], ir[:])
                        cc = hc.tile([32, 1], F32, tag="cc")
                        nc.vector.scalar_tensor_tensor(
                            cc[:], den[:], -2.0, rho[:],
                            mybir.AluOpType.mult, mybir.AluOpType.mult)
                        dx = hc.tile([32, 56], F32, tag="dx")
                        nc.vector.tensor_scalar(dx[:], negio[:], mx[:, 0:1], None,
                                                mybir.AluOpType.add)
                        dy = hc.tile([32, 56], F32, tag="dy")
                        nc.vector.tensor_scalar(dy[:], negio[:], my[:, 0:1], None,
                                                mybir.AluOpType.add)
                        u = hc.tile([32, 56], F32, tag="u")
                        nc.vector.scalar_tensor_tensor(
                            u[:], dx[:], ai[:, 0:1], dx[:],
                            mybir.AluOpType.mult, mybir.AluOpType.mult)
                        v = hc.tile([32, 56], F32, tag="v")
                        nc.vector.scalar_tensor_tensor(
                            v[:], dy[:], bj[:, 0:1], dy[:],
                            mybir.AluOpType.mult, mybir.AluOpType.mult)
                        lt = attp.tile([32, 56, 56], F32, tag="lt")
                        nc.vector.scalar_tensor_tensor(
                            lt[:], dx[:].unsqueeze(2).broadcast_to([32, 56, 56]),
                            cc[:, 0:1],
                            dy[:].unsqueeze(1).broadcast_to([32, 56, 56]),
                            mybir.AluOpType.mult, mybir.AluOpType.mult)
                        nc.vector.tensor_add(
                            lt[:], lt[:],
                            u[:].unsqueeze(2).broadcast_to([32, 56, 56]))
                        nc.vector.tensor_add(
                            lt[:], lt[:],
                            v[:].unsqueeze(1).broadcast_to([32, 56, 56]))
                        att = attp.tile([32, 56 * 56], F32, tag="att")
                        asum = hc.tile([32, 1], F32, tag="asum")
                        nc.scalar.activation(
                            att[:], lt[:].rearrange("p a b -> p (a b)"),
                            mybir.ActivationFunctionType.Exp,
                            accum_out=asum[:])
                        inv = hc.tile([32, 1], F32, tag="inv")
                        nc.vector.reciprocal(inv[:], asum[:])
                        nc.vector.tensor_scalar(att[:], att[:], inv[:, 0:1], None,
                                                mybir.AluOpType.mult)
                        obuf = attp.tile([8, 56 * 56], F32, tag="obuf")
                        for ch in range(7):
                            pso = psc.tile([8, 448], F32, tag="pso")
                            nc.tensor.matmul(pso[:], selg[:],
                                             att[:, ch * 448:(ch + 1) * 448],
                                             start=True, stop=True)
                            nc.vector.tensor_copy(
                                obuf[:, ch * 448:(ch + 1) * 448], pso[:])
                        nc.gpsimd.dma_start(
                            out[img].rearrange("o a b -> o (a b)"), obuf[:])

        if r_loop:
            with tc.For_i(0, r_loop, 1):
                emit_body()
        else:
            emit_body()
    nc.compile()
    return nc


def prep_inputs(inputs, split1=False, split2=False):
    import ml_dtypes
    E4 = ml_dtypes.float8_e4m3
    x = inputs["x"]
    eps_s = 1.0 / np.sqrt(np.float32(1.0 + 1e-5))

    def fold(w, g):
        s = (np.asarray(g, np.float32) * eps_s).astype(np.float32)
        return (np.asarray(w, np.float32) * s[:, None, None, None])

    w1 = fold(inputs["w1"], inputs["g1"])
    w2 = fold(inputs["w2"], inputs["g2"])
    w3 = fold(inputs["w3"], inputs["g3"]) / 256.0
    w4 = fold(inputs["w4"], inputs["g4"])
    w5 = fold(inputs["w5"], inputs["g5"]) / 9.0
    wfc = np.asarray(inputs["w_fc"], np.float32)
    mw = np.asarray(inputs["mix_w"], np.float32).reshape(OUT, GMM)
    mw = np.exp(mw - mw.max(1, keepdims=True))
    mw = mw / mw.sum(1, keepdims=True)

    def wt_layout(w, ncin):
        co = w.shape[0]
        r = w.transpose(1, 2, 3, 0).reshape(ncin, 128, 9, co)
        return np.ascontiguousarray(r.transpose(1, 2, 0, 3))  # [128,9,ncin,co]

    def q8_layout(wf, ncin, split):
        ws = wt_layout(wf, ncin) * WS                      # [128,9,ncin,co]
        hi = ws.astype(E4)
        if not split:
            return np.ascontiguousarray(hi[:, None])       # [128,1,9,ncin,co]
        lo = ((ws - hi.astype(np.float32)) * RSPLIT).astype(E4)
        return np.ascontiguousarray(np.stack([hi, lo], axis=1))

    w1t = q8_layout(w1, 4, split1)
    w2t = q8_layout(w2, 2, split2)
    w3t = wt_layout(w3.astype(np.float32), 2)
    w4t = wt_layout(w4.astype(np.float32), 1)[:, :, 0, :]
    w5t = wt_layout(w5.astype(np.float32), 1)[:, :, 0, :]
    wfct = np.ascontiguousarray(wfc.reshape(128, 64, 9).transpose(1, 2, 0))

    def bias_chunks(b, nchunk):
        return np.ascontiguousarray(
            np.asarray(b, np.float32).reshape(nchunk, 128).T)

    b1h = bias_chunks(inputs["b1"], 2)
    b2h = bias_chunks(inputs["b2"], 2)
    b3h = np.asarray(inputs["b3"], np.float32).reshape(128, 1)
    b4h = np.asarray(inputs["b4"], np.float32).reshape(128, 1)
    b5h = np.asarray(inputs["b5"], np.float32).reshape(64, 1)

    selp = np.zeros((128, 128), np.float32)
    for m in range(32):
        selp[4 * m + 0, m] = float(H2 - 1)
        selp[4 * m + 1, m + 32] = float(W2 - 1)
        selp[4 * m + 2, m + 64] = 2.0 * LOGR
        selp[4 * m + 3, m + 96] = 1.6
    selg = np.zeros((32, 8), np.float32)
    for o in range(OUT):
        for g in range(GMM):
            selg[o * GMM + g, o] = mw[o, g]
    negio = np.broadcast_to(-np.arange(56, dtype=np.float32), (32, 56)).copy()
    cst = np.full((32, 1), -LOGR, np.float32)

    xp = np.zeros((B, 4, 128, HP, WP), E4)
    xp[:, :, :, 1:113, 1:113] = np.asarray(x, np.float32).reshape(
        B, 4, 128, H, W).astype(E4)
    xp = xp.reshape(B, 4, 128, FLAT)

    common = {
        "w1t": w1t, "w2t": w2t,
        "w3t": w3t, "w4t": w4t, "w5t": w5t, "wfct": wfct,
        "b1d": b1h, "b2d": b2h, "b3d": b3h, "b4d": b4h, "b5d": b5h,
        "selpd": selp, "selgd": selg, "negiod": negio, "cstd": cst,
    }
    in_maps = []
    for c in range(NCORE):
        m = dict(common)
        m["x"] = np.ascontiguousarray(xp[c * IMG:(c + 1) * IMG])
        in_maps.append(m)
    return in_maps


from concourse.bass_utils import run_bass_kernel_spmd

SPLIT1 = False
SPLIT2 = True

_CACHE = {}


def kernel(**inputs):
    inputs = {k: np.asarray(v) for k, v in inputs.items()}
    if "nc" not in _CACHE:
        _CACHE["nc"] = build_nc(split1=SPLIT1, split2=SPLIT2)
    nc = _CACHE["nc"]
    in_maps = prep_inputs(inputs, SPLIT1, SPLIT2)
    res = run_bass_kernel_spmd(nc, in_maps, core_ids=list(range(NCORE)))
    out = np.concatenate([res.results[c]["out"] for c in range(NCORE)], axis=0)
    return np.ascontiguousarray(out.astype(np.float32))


# revision 9
# speedup vs baseline: 1.1686x; 1.1686x over previous
"""Trainium2 Bass kernel for nn_BivariateNormalAttention (fp8 DoubleRow).

Self-contained: takes FULL inputs (B=16), shards batch across 8 NeuronCores
(2 images/core), runs a Bass/Tile kernel per core, gathers [16,8,56,56].

conv1 (512->256) runs in fp8e4m3 with perf_mode=DoubleRow (2x128-ch
contraction per pass, 0.5 cyc/row); conv2 (256->256) the same but with
hi+lo split weights (residual x16) to keep its quantization error out of
the pooled features. Convs 3-5 / fc / attention head run in fp32 exactly
as the f32r baseline. Weights are prescaled by 64 on the host to dodge
fp8 subnormals; the activation stage rescales. conv1 output stays
SBUF-resident in fp8 (no DRAM roundtrip). 8 PSUM banks accumulate 8
strips (4 for split) per LDWEIGHTS so weight loads amortize.
"""
import sys
import numpy as np

for _p in ("/opt/trn_rl_repo", "/root/.axon_site/_ro/trn_rl_repo"):
    if _p not in sys.path:
        sys.path.append(_p)

import concourse.bacc as bacc
import concourse.mybir as mybir
import concourse.tile as tile


F32 = mybir.dt.float32
FP8 = mybir.dt.float8e4
DR = mybir.MatmulPerfMode.DoubleRow

B, C, H, W = 16, 512, 112, 112
OUT, GMM = 8, 4
NCORE = 8
IMG = B // NCORE
HP, WP = H + 2, W + 2            # 114
FLAT = HP * WP                   # 12996
FLATP = 13008                    # FLAT padded to %16
RS = 2                           # strip rows (DR moving limit: 2*228 <= 512)
NSTRIP = H // RS                 # 56
NFREE = RS * WP                  # 228
H2 = W2 = H // 2
SIG2 = float(H) / 2.0
LOGR = float(np.log(3.0))
WS = 64.0                        # weight prescale
RSPLIT = 16.0                    # residual prescale for split weights


def build_nc(r_loop=None, split1=False, split2=False, phases="abc"):
    nc = bacc.Bacc("TRN2", target_bir_lowering=False, debug=False)

    n1 = 2 if split1 else 1
    n2 = 2 if split2 else 1
    x = nc.dram_tensor("x", [IMG, 4, 128, FLAT], FP8, kind="ExternalInput")
    w1t = nc.dram_tensor("w1t", [128, n1, 9, 4, 256], FP8, kind="ExternalInput")
    w2t = nc.dram_tensor("w2t", [128, n2, 9, 2, 256], FP8, kind="ExternalInput")
    w3t = nc.dram_tensor("w3t", [128, 9, 2, 128], F32, kind="ExternalInput")
    w4t = nc.dram_tensor("w4t", [128, 9, 128], F32, kind="ExternalInput")
    w5t = nc.dram_tensor("w5t", [128, 9, 64], F32, kind="ExternalInput")
    wfct = nc.dram_tensor("wfct", [64, 9, 128], F32, kind="ExternalInput")
    b1d = nc.dram_tensor("b1d", [128, 2], F32, kind="ExternalInput")
    b2d = nc.dram_tensor("b2d", [128, 2], F32, kind="ExternalInput")
    b3d = nc.dram_tensor("b3d", [128, 1], F32, kind="ExternalInput")
    b4d = nc.dram_tensor("b4d", [128, 1], F32, kind="ExternalInput")
    b5d = nc.dram_tensor("b5d", [64, 1], F32, kind="ExternalInput")
    selpd = nc.dram_tensor("selpd", [128, 128], F32, kind="ExternalInput")
    selgd = nc.dram_tensor("selgd", [32, 8], F32, kind="ExternalInput")
    negiod = nc.dram_tensor("negiod", [32, 56], F32, kind="ExternalInput")
    cstd = nc.dram_tensor("cstd", [32, 1], F32, kind="ExternalInput")

    out = nc.dram_tensor("out", [IMG, OUT, H2, W2], F32, kind="ExternalOutput")

    GA = 8 // n1                  # strips per weight-load group, conv1
    GB = 8 // n2                  # strips per group, conv2
    GLA = (2 * GA + 2) * WP + 2   # group x-span (elements)
    GLA16 = (GLA + 15) // 16 * 16
    inv_ws = 1.0 / WS
    inv_rs = 1.0 / RSPLIT

    with tile.TileContext(nc) as tc:
        def emit_body():
            with (
                tc.tile_pool(name="persist", bufs=1) as pp,
            ):
                b1 = pp.tile([128, 2], F32)
                b2 = pp.tile([128, 2], F32)
                nc.gpsimd.dma_start(b1[:], b1d[:])
                nc.gpsimd.dma_start(b2[:], b2d[:])
                pacc = [[pp.tile([128, 49], F32, name=f"pacc{i}_{c}", tag=f"pacc{i}_{c}")
                         for c in range(2)] for i in range(IMG)]
                # SBUF-resident conv1 output, fp8, per image
                c1t = [pp.tile([128, 2, FLATP], FP8, name=f"c1_{i}", tag=f"c1_{i}")
                       for i in range(IMG)]
                # zero borders + flat tail once (gpsimd, overlaps compute)
                for i in range(IMG):
                    for co in range(2):
                        v = c1t[i][:, co, 0:FLAT].rearrange(
                            "p (r c) -> p r c", c=WP)
                        nc.gpsimd.memset(v[:, 0:1, :], 0.0)
                        nc.gpsimd.memset(v[:, HP - 1:HP, :], 0.0)
                        nc.gpsimd.memset(v[:, :, 0:1], 0.0)
                        nc.gpsimd.memset(v[:, :, WP - 1:WP], 0.0)
                        nc.gpsimd.memset(c1t[i][:, co, FLAT:FLATP], 0.0)

                # ---------------- phase A: conv1 ----------------
                with (
                    tc.tile_pool(name="w1p", bufs=1) as w1p,
                    tc.tile_pool(name="xa", bufs=3) as xa,
                    tc.tile_pool(name="psa", bufs=1, space="PSUM") as psa,
                ):
                    w1 = w1p.tile([128, n1, 9, 4, 256], FP8)
                    nc.gpsimd.dma_start(w1[:], w1t[:])
                    for img in (range(IMG) if "a" in phases else []):
                        xflat = x[img]
                        for sg in range(NSTRIP // GA):
                            base = RS * GA * sg * WP
                            nload = min(GLA, FLAT - base)
                            xt = xa.tile([128, 4, GLA16], FP8, tag="xg")
                            if nload < GLA16:
                                nc.vector.memset(xt[:, :, nload:GLA16], 0.0)
                            for ci in range(4):
                                nc.gpsimd.dma_start(
                                    xt[:, ci, :nload],
                                    xflat[ci, :, base:base + nload])
                            for co in range(2):
                                pss = [psa.tile([128, NFREE], F32,
                                                name=f"psA{q}", tag=f"psA{q}")
                                       for q in range(8)]
                                k = 0
                                for p in range(2):
                                    for t in range(9):
                                        toff = (t // 3) * WP + t % 3
                                        for sp in range(n1):
                                            lh = w1[:, sp, t, 2 * p:2 * p + 2,
                                                    co * 128:(co + 1) * 128]
                                            for g in range(GA):
                                                mi = nc.tensor.matmul(
                                                    pss[sp * GA + g][:], lh,
                                                    xt[:, 2 * p:2 * p + 2,
                                                       RS * g * WP + toff:
                                                       RS * g * WP + toff + NFREE],
                                                    start=(k == 0),
                                                    stop=(k == 17),
                                                    perf_mode=DR)
                                        k += 1
                                for g in range(GA):
                                    if split1:
                                        ea = xa.tile([128, NFREE], F32,
                                                     tag="ea")
                                        nc.vector.tensor_copy(
                                            ea[:], pss[GA + g][:])
                                        nc.vector.scalar_tensor_tensor(
                                            pss[g][:], ea[:], inv_rs,
                                            pss[g][:], mybir.AluOpType.mult,
                                            mybir.AluOpType.add)
                                    s_abs = GA * sg + g
                                    vdst = c1t[img][:, co, 0:FLAT].rearrange(
                                        "p (r c) -> p r c", c=WP)
                                    nc.scalar.activation(
                                        vdst[:, 1 + RS * s_abs:1 + RS * (s_abs + 1),
                                             1:113],
                                        pss[g][:].rearrange(
                                            "p (a b) -> p a b", b=WP)[:, :, 0:112],
                                        mybir.ActivationFunctionType.Relu,
                                        bias=b1[:, co:co + 1], scale=inv_ws)

                # ---------------- phase B: conv2 + avgpool16 ----------------
                with (
                    tc.tile_pool(name="w2p", bufs=1) as w2p,
                    tc.tile_pool(name="ob", bufs=4) as ob,
                    tc.tile_pool(name="psb", bufs=1, space="PSUM") as psb,
                ):
                    w2 = w2p.tile([128, n2, 9, 2, 256], FP8)
                    nc.gpsimd.dma_start(w2[:], w2t[:])
                    for img in (range(IMG) if "b" in phases else []):
                        for c in range(2):
                            nc.vector.memset(pacc[img][c][:], 0.0)
                        for sg in range(NSTRIP // GB):
                            for co in range(2):
                                pss = [psb.tile([128, NFREE], F32,
                                                name=f"psB{q}", tag=f"psB{q}")
                                       for q in range(8)]
                                for t in range(9):
                                    toff = (t // 3) * WP + t % 3
                                    for sp in range(n2):
                                        lh = w2[:, sp, t, 0:2,
                                                co * 128:(co + 1) * 128]
                                        for g in range(GB):
                                            s_abs = GB * sg + g
                                            off = RS * s_abs * WP + toff
                                            mi = nc.tensor.matmul(
                                                pss[sp * GB + g][:], lh,
                                                c1t[img][:, 0:2,
                                                         off:off + NFREE],
                                                start=(t == 0), stop=(t == 8),
                                                perf_mode=DR)
                                for g in range(GB):
                                    if split2:
                                        el = ob.tile([128, NFREE], F32,
                                                     tag="el")
                                        nc.vector.tensor_copy(
                                            el[:], pss[GB + g][:])
                                        nc.vector.scalar_tensor_tensor(
                                            pss[g][:], el[:], inv_rs,
                                            pss[g][:], mybir.AluOpType.mult,
                                            mybir.AluOpType.add)
                                    s_abs = GB * sg + g
                                    et = ob.tile([128, RS, WP], F32, tag="et")
                                    nc.scalar.activation(
                                        et[:],
                                        pss[g][:].rearrange(
                                            "p (a b) -> p a b", b=WP),
                                        mybir.ActivationFunctionType.Relu,
                                        bias=b2[:, co:co + 1], scale=inv_ws)
                                    rs_ = ob.tile([128, 7], F32, tag="rs")
                                    nc.vector.reduce_sum(
                                        rs_[:],
                                        et[:, :, 0:112].rearrange(
                                            "p r (g c) -> p g r c", c=16),
                                        axis=mybir.AxisListType.XY)
                                    blk = RS * s_abs // 16
                                    nc.vector.tensor_add(
                                        pacc[img][co][:, blk * 7:(blk + 1) * 7],
                                        pacc[img][co][:, blk * 7:(blk + 1) * 7],
                                        rs_[:])

                # ---------------- phase C: head (same as baseline) ----------
                with (
                    tc.tile_pool(name="wc", bufs=1) as wc,
                    tc.tile_pool(name="hc", bufs=1) as hc,
                    tc.tile_pool(name="att", bufs=1) as attp,
                    tc.tile_pool(name="psc", bufs=1, space="PSUM") as psc,
                ):
                    w3 = wc.tile([128, 9, 2, 128], F32)
                    w4 = wc.tile([128, 9, 128], F32)
                    w5 = wc.tile([128, 9, 64], F32)
                    wfc = wc.tile([64, 9, 128], F32)
                    b3 = wc.tile([128, 1], F32)
                    b4 = wc.tile([128, 1], F32)
                    b5 = wc.tile([64, 1], F32)
                    selp = wc.tile([128, 128], F32)
                    selg = wc.tile([32, 8], F32)
                    negio = wc.tile([32, 56], F32)
                    cst = wc.tile([32, 1], F32)
                    for tdst, tsrc in ((w3, w3t), (w4, w4t), (w5, w5t),
                                       (wfc, wfct), (b3, b3d), (b4, b4d),
                                       (b5, b5d), (selp, selpd), (selg, selgd),
                                       (negio, negiod), (cst, cstd)):
                        nc.gpsimd.dma_start(tdst[:], tsrc[:])

                    for img in (range(IMG) if "c" in phases else []):
                        p3in = []
                        for ci in range(2):
                            pi = hc.tile([128, 83], F32, tag=f"p3in{ci}")
                            nc.vector.memset(pi[:], 0.0)
                            nc.vector.tensor_copy(
                                pi[:, 10:73].rearrange(
                                    "p (a b) -> p a b", b=9)[:, :, 0:7],
                                pacc[img][ci][:].rearrange(
                                    "p (a b) -> p a b", b=7))
                            p3in.append(pi)
                        ps3 = psc.tile([128, 63], F32, tag="ps3")
                        k = 0
                        for ci in range(2):
                            for t in range(9):
                                nc.tensor.matmul(
                                    ps3[:], w3[:, t, ci, :],
                                    p3in[ci][:, (t // 3) * 9 + t % 3:
                                             (t // 3) * 9 + t % 3 + 63],
                                    start=(k == 0), stop=(k == 17))
                                k += 1
                        p4in = hc.tile([128, 83], F32, tag="p4in")
                        nc.vector.memset(p4in[:], 0.0)
                        nc.scalar.activation(
                            p4in[:, 10:73].rearrange(
                                "p (a b) -> p a b", b=9)[:, :, 0:7],
                            ps3[:].rearrange("p (a b) -> p a b", b=9)[:, :, 0:7],
                            mybir.ActivationFunctionType.Relu, bias=b3[:, 0:1])
                        ps4 = psc.tile([128, 63], F32, tag="ps4")
                        for t in range(9):
                            nc.tensor.matmul(
                                ps4[:], w4[:, t, :],
                                p4in[:, (t // 3) * 9 + t % 3:
                                     (t // 3) * 9 + t % 3 + 63],
                                start=(t == 0), stop=(t == 8))
                        c4t = hc.tile([128, 49], F32, tag="c4t")
                        nc.scalar.activation(
                            c4t[:].rearrange("p (a b) -> p a b", b=7),
                            ps4[:].rearrange("p (a b) -> p a b", b=9)[:, :, 0:7],
                            mybir.ActivationFunctionType.Relu, bias=b4[:, 0:1])
                        c4v = c4t[:].rearrange("p (y x) -> p y x", x=7)
                        a1 = hc.tile([128, 7, 3], F32, tag="a1")
                        nc.vector.tensor_add(a1[:], c4v[:, :, 0:5:2],
                                             c4v[:, :, 1:6:2])
                        nc.vector.tensor_add(a1[:], a1[:], c4v[:, :, 2:7:2])
                        a2 = hc.tile([128, 9], F32, tag="a2")
                        a2v = a2[:].rearrange("p (i j) -> p i j", j=3)
                        nc.vector.tensor_add(a2v, a1[:, 0:5:2, :],
                                             a1[:, 1:6:2, :])
                        nc.vector.tensor_add(a2v, a2v, a1[:, 2:7:2, :])
                        p5in = hc.tile([128, 27], F32, tag="p5in")
                        nc.vector.memset(p5in[:], 0.0)
                        nc.vector.tensor_copy(
                            p5in[:, 6:21].rearrange(
                                "p (a b) -> p a b", b=5)[:, :, 0:3],
                            a2[:].rearrange("p (a b) -> p a b", b=3))
                        ps5 = psc.tile([64, 15], F32, tag="ps5")
                        for t in range(9):
                            nc.tensor.matmul(
                                ps5[:], w5[:, t, :],
                                p5in[:, (t // 3) * 5 + t % 3:
                                     (t // 3) * 5 + t % 3 + 15],
                                start=(t == 0), stop=(t == 8))
                        h5 = hc.tile([64, 9], F32, tag="h5")
                        nc.scalar.activation(
                            h5[:].rearrange("p (a b) -> p a b", b=3),
                            ps5[:].rearrange("p (a b) -> p a b", b=5)[:, :, 0:3],
                            mybir.ActivationFunctionType.Relu, bias=b5[:, 0:1])
                        psf = psc.tile([128, 1], F32, tag="psf")
                        for t in range(9):
                            nc.tensor.matmul(psf[:], wfc[:, t, :], h5[:, t:t + 1],
                                             start=(t == 0), stop=(t == 8))
                        sig = hc.tile([128, 1], F32, tag="sig")
                        nc.scalar.activation(sig[:], psf[:],
                                             mybir.ActivationFunctionType.Sigmoid)
                        psl = psc.tile([128, 1], F32, tag="psl")
                        nc.tensor.matmul(psl[:], selp[:], sig[:],
                                         start=True, stop=True)
                        mx = hc.tile([32, 1], F32, tag="mx")
                        my = hc.tile([32, 1], F32, tag="my")
                        nc.vector.tensor_copy(mx[:], psl[0:32])
                        nc.vector.tensor_copy(my[:], psl[32:64])
                        r32 = hc.tile([32, 1], F32, tag="r32")
                        nc.scalar.activation(r32[:], psl[64:96],
                                             mybir.ActivationFunctionType.Exp,
                                             bias=cst[:, 0:1])
                        rho = hc.tile([32, 1], F32, tag="rho")
                        nc.vector.tensor_scalar(rho[:], psl[96:128], -0.8, None,
                                                mybir.AluOpType.add)
                        rr = hc.tile([32, 1], F32, tag="rr")
                        nc.vector.tensor_mul(rr[:], rho[:], rho[:])
                        om = hc.tile([32, 1], F32, tag="om")
                        nc.vector.tensor_scalar(om[:], rr[:], -1.0, 1.0,
                                                mybir.AluOpType.mult,
                                                mybir.AluOpType.add)
                        iom = hc.tile([32, 1], F32, tag="iom")
                        nc.vector.reciprocal(iom[:], om[:])
                        den = hc.tile([32, 1], F32, tag="den")
                        nc.vector.tensor_scalar(den[:], iom[:],
                                                -0.5 / (SIG2 * SIG2), None,
                                                mybir.AluOpType.mult)
                        ai = hc.tile([32, 1], F32, tag="ai")
                        nc.vector.tensor_mul(ai[:], den[:], r32[:])
                        ir = hc.tile([32, 1], F32, tag="ir")
                        nc.vector.reciprocal(ir[:], r32[:])
                        bj = hc.tile([32, 1], F32, tag="bj")
                        nc.vector.tensor_mul(bj[:], den[:], ir[:])
                        cc = hc.tile([32, 1], F32, tag="cc")
                        nc.vector.scalar_tensor_tensor(
                            cc[:], den[:], -2.0, rho[:],
                            mybir.AluOpType.mult, mybir.AluOpType.mult)
                        dx = hc.tile([32, 56], F32, tag="dx")
                        nc.vector.tensor_scalar(dx[:], negio[:], mx[:, 0:1], None,
                                                mybir.AluOpType.add)
                        dy = hc.tile([32, 56], F32, tag="dy")
                        nc.vector.tensor_scalar(dy[:], negio[:], my[:, 0:1], None,
                                                mybir.AluOpType.add)
                        u = hc.tile([32, 56], F32, tag="u")
                        nc.vector.scalar_tensor_tensor(
                            u[:], dx[:], ai[:, 0:1], dx[:],
                            mybir.AluOpType.mult, mybir.AluOpType.mult)
                        v = hc.tile([32, 56], F32, tag="v")
                        nc.vector.scalar_tensor_tensor(
                            v[:], dy[:], bj[:, 0:1], dy[:],
                            mybir.AluOpType.mult, mybir.AluOpType.mult)
                        lt = attp.tile([32, 56, 56], F32, tag="lt")
                        nc.vector.scalar_tensor_tensor(
                            lt[:], dx[:].unsqueeze(2).broadcast_to([32, 56, 56]),
                            cc[:, 0:1],
                            dy[:].unsqueeze(1).broadcast_to([32, 56, 56]),
                            mybir.AluOpType.mult, mybir.AluOpType.mult)
                        nc.gpsimd.tensor_add(
                            lt[:], lt[:],
                            u[:].unsqueeze(2).broadcast_to([32, 56, 56]))
                        nc.gpsimd.tensor_add(
                            lt[:], lt[:],
                            v[:].unsqueeze(1).broadcast_to([32, 56, 56]))
                        att = attp.tile([32, 56 * 56], F32, tag="att")
                        asum = hc.tile([32, 1], F32, tag="asum")
                        nc.scalar.activation(
                            att[:], lt[:].rearrange("p a b -> p (a b)"),
                            mybir.ActivationFunctionType.Exp,
                            accum_out=asum[:])
                        inv = hc.tile([32, 1], F32, tag="inv")
                        nc.vector.reciprocal(inv[:], asum[:])
                        selgs = hc.tile([32, 8], F32, tag="selgs")
                        nc.vector.tensor_scalar(selgs[:], selg[:], inv[:, 0:1],
                                                None, mybir.AluOpType.mult)
                        obuf = attp.tile([8, 56 * 56], F32, tag="obuf")
                        for ch in range(7):
                            pso = psc.tile([8, 448], F32, tag="pso")
                            nc.tensor.matmul(pso[:], selgs[:],
                                             att[:, ch * 448:(ch + 1) * 448],
                                             start=True, stop=True)
                            nc.scalar.copy(
                                obuf[:, ch * 448:(ch + 1) * 448], pso[:])
                        nc.gpsimd.dma_start(
                            out[img].rearrange("o a b -> o (a b)"), obuf[:])

        if r_loop:
            with tc.For_i(0, r_loop, 1):
                emit_body()
        else:
            emit_body()
    nc.compile()
    return nc


def prep_inputs(inputs, split1=False, split2=False):
    import ml_dtypes
    E4 = ml_dtypes.float8_e4m3
    x = inputs["x"]
    eps_s = 1.0 / np.sqrt(np.float32(1.0 + 1e-5))

    def fold(w, g):
        s = (np.asarray(g, np.float32) * eps_s).astype(np.float32)
        return (np.asarray(w, np.float32) * s[:, None, None, None])

    w1 = fold(inputs["w1"], inputs["g1"])
    w2 = fold(inputs["w2"], inputs["g2"])
    w3 = fold(inputs["w3"], inputs["g3"]) / 256.0
    w4 = fold(inputs["w4"], inputs["g4"])
    w5 = fold(inputs["w5"], inputs["g5"]) / 9.0
    wfc = np.asarray(inputs["w_fc"], np.float32)
    mw = np.asarray(inputs["mix_w"], np.float32).reshape(OUT, GMM)
    mw = np.exp(mw - mw.max(1, keepdims=True))
    mw = mw / mw.sum(1, keepdims=True)

    def wt_layout(w, ncin):
        co = w.shape[0]
        r = w.transpose(1, 2, 3, 0).reshape(ncin, 128, 9, co)
        return np.ascontiguousarray(r.transpose(1, 2, 0, 3))  # [128,9,ncin,co]

    def q8_layout(wf, ncin, split):
        ws = wt_layout(wf, ncin) * WS                      # [128,9,ncin,co]
        hi = ws.astype(E4)
        if not split:
            return np.ascontiguousarray(hi[:, None])       # [128,1,9,ncin,co]
        lo = ((ws - hi.astype(np.float32)) * RSPLIT).astype(E4)
        return np.ascontiguousarray(np.stack([hi, lo], axis=1))

    w1t = q8_layout(w1, 4, split1)
    w2t = q8_layout(w2, 2, split2)
    w3t = wt_layout(w3.astype(np.float32), 2)
    w4t = wt_layout(w4.astype(np.float32), 1)[:, :, 0, :]
    w5t = wt_layout(w5.astype(np.float32), 1)[:, :, 0, :]
    wfct = np.ascontiguousarray(wfc.reshape(128, 64, 9).transpose(1, 2, 0))

    def bias_chunks(b, nchunk):
        return np.ascontiguousarray(
            np.asarray(b, np.float32).reshape(nchunk, 128).T)

    b1h = bias_chunks(inputs["b1"], 2)
    b2h = bias_chunks(inputs["b2"], 2)
    b3h = np.asarray(inputs["b3"], np.float32).reshape(128, 1)
    b4h = np.asarray(inputs["b4"], np.float32).reshape(128, 1)
    b5h = np.asarray(inputs["b5"], np.float32).reshape(64, 1)

    selp = np.zeros((128, 128), np.float32)
    for m in range(32):
        selp[4 * m + 0, m] = float(H2 - 1)
        selp[4 * m + 1, m + 32] = float(W2 - 1)
        selp[4 * m + 2, m + 64] = 2.0 * LOGR
        selp[4 * m + 3, m + 96] = 1.6
    selg = np.zeros((32, 8), np.float32)
    for o in range(OUT):
        for g in range(GMM):
            selg[o * GMM + g, o] = mw[o, g]
    negio = np.broadcast_to(-np.arange(56, dtype=np.float32), (32, 56)).copy()
    cst = np.full((32, 1), -LOGR, np.float32)

    xp = np.zeros((B, 4, 128, HP, WP), E4)
    xp[:, :, :, 1:113, 1:113] = np.asarray(x, np.float32).reshape(
        B, 4, 128, H, W).astype(E4)
    xp = xp.reshape(B, 4, 128, FLAT)

    common = {
        "w1t": w1t, "w2t": w2t,
        "w3t": w3t, "w4t": w4t, "w5t": w5t, "wfct": wfct,
        "b1d": b1h, "b2d": b2h, "b3d": b3h, "b4d": b4h, "b5d": b5h,
        "selpd": selp, "selgd": selg, "negiod": negio, "cstd": cst,
    }
    in_maps = []
    for c in range(NCORE):
        m = dict(common)
        m["x"] = np.ascontiguousarray(xp[c * IMG:(c + 1) * IMG])
        in_maps.append(m)
    return in_maps


from concourse.bass_utils import run_bass_kernel_spmd

SPLIT1 = False
SPLIT2 = True

_CACHE = {}


def kernel(**inputs):
    inputs = {k: np.asarray(v) for k, v in inputs.items()}
    if "nc" not in _CACHE:
        _CACHE["nc"] = build_nc(split1=SPLIT1, split2=SPLIT2)
    nc = _CACHE["nc"]
    in_maps = prep_inputs(inputs, SPLIT1, SPLIT2)
    res = run_bass_kernel_spmd(nc, in_maps, core_ids=list(range(NCORE)))
    out = np.concatenate([res.results[c]["out"] for c in range(NCORE)], axis=0)
    return np.ascontiguousarray(out.astype(np.float32))
